# revision 4
# baseline (speedup 1.0000x reference)
"""Trainium2 Bass kernel for the Expected-Depth DP loss.

Computation (see reference):
  - edge_max = max over first 7 of 8 op-logits          [S, 64, 16]
  - w        = masked softmax over the 16-wide window   [S, 64, 16]
  - DP scan:  ed[j] = sum_k w[j,k] * (ed[base+k] + 1),  j = 2..65
  - loss     = sum_s theta[s] * softmax(beta[s]) . (ed[ii] + ed[jj])

Sharding: S=8192 stages split across 8 cores (pure data parallel,
1024 stages/core). Per-core partial losses are summed on the host.

v2 layout: alpha is host-transposed into 8 node-slabs; slab i holds
nodes [8i, 8i+8) of all 1024 stages, with each SBUF partition reading
one contiguous 32KB chunk. Slabs are cast fp32->bf16 during the SWDGE
DMA. After slab i is reduced to softmax weights, DP steps j=8i+2..
8i+9 run over all 8 stage-tiles, so the sequential scan streams along
with the DMA instead of trailing it. All 16 input DMAs share the
single SWDGE FIFO in [alpha..., beta...] order so beta never steals
alpha bandwidth; beta matmul/dot work then fills the tail.

Engines: DVE does the bf16 max tree + additive window mask + group
sums + reciprocal + DP + final dots; ACT does exp(mx), exp(beta) and
the PSUM->SBUF copies; gpsimd issues the cast-DMAs and the softmax
normalize multiply; PE does the beta incidence matmuls.
"""

import numpy as np

SW = 16          # DP window
NN = 64          # nodes per stage
NOPS = 8         # ops per edge (last excluded from the max)
S = 8192         # stages
E = 2016         # beta edges
P = 128          # SBUF partitions
N_CORES = 8
S_CORE = S // N_CORES        # 1024
T = S_CORE // P              # 8 stage-tiles per core
NSLAB = 8                    # node slabs
NPS = NN // NSLAB            # 8 nodes per slab
SLABW = T * NPS * SW         # 1024 mx elems per slab (per partition)
AWS_ = SLABW * NOPS          # 8192 alpha floats per slab per partition
EDW = 67                     # ed row stride (66 node slots + 1 pad)
NCH = 16                     # beta column chunks
ECH = E // NCH               # 126 edges per chunk
MNEG = -40.0                 # additive mask for invalid window rows

_CACHE = {}


def _host_consts():
    import ml_dtypes

    ii, jj = [], []
    for i in range(2, NN + 1):
        for j in range(i + 1, NN + 2):
            ii.append(i)
            jj.append(j)
    ii = np.asarray(ii)
    jj = np.asarray(jj)
    # incidence matrix chunks: mt[e_local, c*67 + k] = [ii==k] + [jj==k],
    # column 66 of each chunk is all ones (softmax denominator)
    mt = np.zeros((NCH, ECH, EDW), np.float32)
    for e in range(E):
        c, el = divmod(e, ECH)
        mt[c, el, ii[e]] += 1.0
        mt[c, el, jj[e]] += 1.0
        mt[c, el, EDW - 1] = 1.0
    mt = np.ascontiguousarray(
        mt.transpose(1, 0, 2).reshape(ECH, NCH * EDW)
    ).astype(ml_dtypes.bfloat16)
    # additive masks for slabs 0/1 (node n: rows k < n+2 valid)
    def neg(nodes):
        m = np.zeros((len(nodes), SW), np.float32)
        for r, n in enumerate(nodes):
            m[r, min(n + 2, SW):] = MNEG
        return np.ascontiguousarray(
            np.broadcast_to(m.reshape(1, -1), (P, m.size))
        ).astype(ml_dtypes.bfloat16)

    mneg0 = neg(range(0, NPS))          # slab 0: nodes 0-7   [P, 128]
    mneg1 = neg(range(NPS, 14))         # slab 1: nodes 8-13  [P, 96]
    return mt, mneg0, mneg1


def _build_nc():
    import concourse.bass as bass
    import concourse.mybir as mybir
    from concourse.tile import TileContext
    from concourse.vector_clock import ScopedClock, VectorClock

    # This walrus build rejects TPB instructions carrying more than one sem
    # wait (two for EventSemaphore), but Tile's wait assignment happily packs
    # 2-3. Split the extras onto single-wait NoOps on the same engine.
    if not getattr(TileContext, "_ant_wait_split", False):
        _orig_commit = TileContext._commit_instruction

        def _commit_split(self, inst, lazy_reg_writes=True):
            si = inst.sync_info
            limit = 2 if isinstance(inst, mybir.InstEventSemaphore) else 1
            if si is not None and si.on_wait and len(si.on_wait) > limit:
                waits = list(si.on_wait)
                for i, w in enumerate(waits[:-limit]):
                    nop = mybir.InstNoOp(
                        name=f"{inst.name}-sw{i}",
                        sync_info=mybir.SyncInfo(on_wait=[w], on_update=[]),
                        bass_nofuse=True,
                        engine=inst.engine,
                    )
                    _orig_commit(self, nop, lazy_reg_writes)
                inst.sync_info = mybir.SyncInfo(
                    on_wait=waits[-limit:], on_update=list(si.on_update)
                )
            return _orig_commit(self, inst, lazy_reg_writes)

        TileContext._commit_instruction = _commit_split
        TileContext._ant_wait_split = True

    # The stock TileContext tail drain packs every outstanding sem wait into
    # a single InstDrain; this walrus caps non-EventSemaphore instructions at
    # one wait. Emit one drain per outstanding semaphore instead.
    def _drain_and_barrier(self, tick_clock, wait_clock):
        nc = self.nc
        gc = tick_clock.global_clock
        n = len(gc)
        for i in range(n):
            t = gc[i]
            if t <= 0:
                continue
            vc = VectorClock([0] * n)
            vc.require_at_least(i, t)
            d = nc.sync.drain()
            wait_clock.add_sem_waits(d.ins, ScopedClock({None: vc}))
        nc.all_engine_barrier()
        assert self.sems is not None
        popped = nc._tile_sem_poison_stack.pop()
        assert popped is self._sem_poison
        nc.clear_and_free_semaphores(list(self.sems.allocated().values()))
        nc.all_engine_barrier()

    TileContext._drain_and_barrier = _drain_and_barrier

    f32 = mybir.dt.float32
    bf16 = mybir.dt.bfloat16
    Alu = mybir.AluOpType
    Act = mybir.ActivationFunctionType
    X = mybir.AxisListType.X

    nc = bass.Bass()
    # alpha host-transposed to slab-major: [slab, p, t, nl, k, o] flattened
    # to [NSLAB, P rows of 8192]; partition p's slab read is 32KB contiguous
    alpha_d = nc.declare_dram_parameter(
        "alpha_s", [NSLAB * P, AWS_], f32, isOutput=False
    )
    # beta pre-transposed on the host into chunk layout:
    # beta_t[el, t*2048 + c*128 + p] = beta[t*128 + p, c*126 + el]
    beta_d = nc.declare_dram_parameter("beta_t", [ECH, T * NCH * P], f32, isOutput=False)
    theta_d = nc.declare_dram_parameter("theta_t", [P, T], f32, isOutput=False)
    mneg0_d = nc.declare_dram_parameter("mneg0", [P, NPS * SW], bf16, isOutput=False)
    mneg1_d = nc.declare_dram_parameter("mneg1", [P, 6 * SW], bf16, isOutput=False)
    mt_d = nc.declare_dram_parameter("mt_c", [ECH, NCH * EDW], bf16, isOutput=False)
    out_d = nc.declare_dram_parameter("loss_part", [1, 1], f32, isOutput=True)

    with TileContext(nc) as tc:
        with (
            tc.tile_pool(name="consts", bufs=1) as cp,
            tc.tile_pool(name="alphap", bufs=3) as ap_pool,
            tc.tile_pool(name="mxp", bufs=2) as mxp,
            tc.tile_pool(name="persist", bufs=1) as pp,
            tc.tile_pool(name="smallp", bufs=4) as sp,
            tc.tile_pool(name="betap", bufs=8) as bp,
            tc.tile_pool(name="ebtp", bufs=2) as ep,
            tc.tile_pool(name="psc", bufs=2, space="PSUM") as psc,
        ):
            # ---- alpha slab DMAs: issue the first 3 up front (bufs=3) ----
            a_sl = []
            for i in range(3):
                a = ap_pool.tile([P, AWS_], bf16, tag="a")
                nc.gpsimd.dma_start(a[:, :], alpha_d[i * P : (i + 1) * P, :])
                a_sl.append(a)

            # consts on the idle HWDGE queue
            mneg0_sb = cp.tile([P, NPS * SW], bf16)
            nc.sync.dma_start(mneg0_sb[:, :], mneg0_d[:, :])
            mneg1_sb = cp.tile([P, 6 * SW], bf16)
            nc.sync.dma_start(mneg1_sb[:, :], mneg1_d[:, :])
            mt_sb = cp.tile([ECH, NCH * EDW], bf16)
            nc.sync.dma_start(mt_sb[:, :], mt_d[:, :])
            theta_sb = cp.tile([P, T], f32)
            nc.sync.dma_start(theta_sb[:, :], theta_d[:, :])
            ones_sb = cp.tile([P, 1], f32)
            nc.vector.memset(ones_sb[:, :], 1.0)

            w_sb = pp.tile([P, NSLAB * SLABW], f32)  # normalized weights, slab-major
            ed_sb = pp.tile([P, T * EDW], f32)       # DP state, zero-init
            tmp_sb = pp.tile([P, T * SW], f32)       # DP step scratch
            acc_sb = pp.tile([P, T], f32)            # per-tile theta*depth
            c_sb = pp.tile([P, T * EDW], f32)        # beta matmul results
            nc.vector.memset(ed_sb[:, :], 0.0)

            ed3 = ed_sb.rearrange("p (t k) -> p t k", t=T)
            tmp3 = tmp_sb.rearrange("p (t k) -> p t k", k=SW)
            w5 = w_sb.rearrange(
                "p (s t n k) -> p s t n k", s=NSLAB, t=T, k=SW
            )

            def dp_steps(j_lo, j_hi):
                for j in range(j_lo, j_hi):
                    n = j - 2
                    wid = min(j, SW)
                    base = j - wid
                    nc.vector.scalar_tensor_tensor(
                        tmp3[:, :, 0:wid],
                        ed3[:, :, base : base + wid],
                        1.0,
                        w5[:, n // NPS, :, n % NPS, 0:wid],
                        Alu.add,
                        Alu.mult,
                    )
                    nc.vector.reduce_sum(
                        ed3[:, :, j : j + 1], tmp3[:, :, 0:wid], axis=X
                    )

            # ---- slab loop ----
            b_t = []
            for i in range(NSLAB):
                a = a_sl[i]
                a3 = a.rearrange("p (g o) -> p g o", o=NOPS)
                # max over ops 0-6: bf16 tree
                m1 = mxp.tile([P, SLABW * 4], bf16, tag="m1")
                m1r = m1.rearrange("p (g o) -> p g o", o=4)
                nc.vector.tensor_max(m1r, a3[:, :, 0:4], a3[:, :, 2:6])
                m2 = mxp.tile([P, SLABW * 2], bf16, tag="m2")
                m2r = m2.rearrange("p (g o) -> p g o", o=2)
                nc.vector.tensor_max(m2r, m1r[:, :, 0:2], m1r[:, :, 2:4])
                mx = mxp.tile([P, SLABW], bf16, tag="mx")
                mxr = mx.rearrange("p (g o) -> p g o", o=1)
                nc.vector.tensor_max(mxr, m2r[:, :, 0:1], m2r[:, :, 1:2])
                nc.vector.tensor_max(mxr, mxr, a3[:, :, 6:7])
                # additive window mask (slabs 0/1 only)
                if i < 2:
                    mn = mneg0_sb if i == 0 else mneg1_sb
                    mw = mn.shape[1]
                    mx3 = mx.rearrange("p (t w) -> p t w", t=T)
                    nc.vector.tensor_add(
                        mx3[:, :, 0:mw],
                        mx3[:, :, 0:mw],
                        mn.rearrange("p (o w) -> p o w", o=1).broadcast_to(
                            (P, T, mw)
                        ),
                    )
                # softmax numerator without max-subtraction (|logits| <~ 6)
                e_sl = w_sb[:, i * SLABW : (i + 1) * SLABW]
                nc.scalar.activation(e_sl, mx[:, :], Act.Exp)
                # group sums + reciprocal
                s_t = sp.tile([P, T * NPS], f32, tag="s")
                nc.vector.reduce_sum(
                    s_t[:, :], e_sl.rearrange("p (n k) -> p n k", k=SW), axis=X
                )
                rs = sp.tile([P, T * NPS], f32, tag="rs")
                nc.vector.reciprocal(rs[:, :], s_t[:, :])
                # normalize on gpsimd (frees DVE for the DP)
                rs_b = rs.rearrange("p (n o) -> p n o", o=1).broadcast_to(
                    (P, T * NPS, SW)
                )
                e3 = e_sl.rearrange("p (n k) -> p n k", k=SW)
                nc.gpsimd.tensor_mul(e3, e3, rs_b)
                # prefetch next alpha slab; after the last, queue all betas
                if i + 3 < NSLAB:
                    a2 = ap_pool.tile([P, AWS_], bf16, tag="a")
                    nc.gpsimd.dma_start(
                        a2[:, :],
                        alpha_d[(i + 3) * P : (i + 4) * P, :],
                    )
                    a_sl.append(a2)
                if i == 4:
                    for t in range(T):
                        b = bp.tile([ECH, NCH * P], bf16, tag="b")
                        nc.gpsimd.dma_start(
                            b[:, :],
                            beta_d[:, t * NCH * P : (t + 1) * NCH * P],
                        )
                        b_t.append(b)
                # DP burst for the PREVIOUS slab (its norm ran during this
                # slab's tree) -- keeps DVE off the norm critical path
                if i > 0:
                    dp_steps(2 + (i - 1) * NPS, 2 + i * NPS)
            dp_steps(2 + (NSLAB - 1) * NPS, NN + 2)

            # ---- beta phase: exp + incidence matmuls + copies ----
            c_ps = psc.tile([P, T * 128], f32, tag="c", bufs=1)
            c3 = c_sb.rearrange("p (t k) -> p t k", t=T)
            for t in range(T):
                eb = ep.tile([ECH, NCH * P], bf16, tag="eb")
                nc.scalar.activation(eb[:, :], b_t[t][:, :], Act.Exp)
                for c in range(NCH):
                    nc.tensor.matmul(
                        c_ps[:, t * 128 : t * 128 + EDW],
                        eb[:, c * P : (c + 1) * P],
                        mt_sb[:, c * EDW : (c + 1) * EDW],
                        start=(c == 0),
                        stop=(c == NCH - 1),
                    )
                nc.scalar.copy(
                    c_sb[:, t * EDW : (t + 1) * EDW],
                    c_ps[:, t * 128 : t * 128 + EDW],
                )

            # ---- final dots: theta * (c . ed) / denom, then reduce ----
            for t in range(T):
                prod = sp.tile([P, EDW - 1], f32, tag="prod")
                q = sp.tile([P, 1], f32, tag="q")
                nc.vector.scalar_tensor_tensor(
                    prod[:, :],
                    ed3[:, t, 0 : EDW - 1],
                    0.0,
                    c3[:, t, 0 : EDW - 1],
                    Alu.add,
                    Alu.mult,
                    accum_out=q[:, :],
                )
                rsb = sp.tile([P, 1], f32, tag="rsb")
                nc.vector.reciprocal(rsb[:, :], c3[:, t, EDW - 1 : EDW])
                nc.vector.scalar_tensor_tensor(
                    acc_sb[:, t : t + 1],
                    q[:, :],
                    rsb[:, :],
                    theta_sb[:, t : t + 1],
                    Alu.mult,
                    Alu.mult,
                )

            # ---- final reduction: 8 cols then 128 partitions ----
            accsum = sp.tile([P, 1], f32, tag="accsum")
            nc.vector.reduce_sum(accsum[:, :], acc_sb[:, :], axis=X)
            out_ps = psc.tile([1, 1], f32, tag="outp", bufs=1)
            nc.tensor.matmul(
                out_ps[:, :], accsum[:, :], ones_sb[:, :], start=True, stop=True
            )
            out_sb = sp.tile([1, 1], f32, tag="outs")
            nc.scalar.copy(out_sb[:, :], out_ps[:, :])
            nc.sync.dma_start(out_d[:, :], out_sb[:, :])

    return nc


def _get_compiled():
    if "nc" not in _CACHE:
        _CACHE["nc"] = _build_nc()
        _CACHE["consts"] = _host_consts()
    return _CACHE["nc"], _CACHE["consts"]


def _in_maps(alpha, beta, theta):
    mt, mneg0, mneg1 = _get_compiled()[1]
    alpha = np.ascontiguousarray(alpha, dtype=np.float32).reshape(S, NN * SW * NOPS)
    beta = np.ascontiguousarray(beta, dtype=np.float32)
    theta = np.ascontiguousarray(theta, dtype=np.float32)
    maps = []
    for c in range(N_CORES):
        sl = slice(c * S_CORE, (c + 1) * S_CORE)
        # [slab, p, t, nl*k*o]: partition p reads 32KB contiguous per slab
        alpha_s = np.ascontiguousarray(
            alpha[sl]
            .reshape(T, P, NSLAB, NPS * SW * NOPS)
            .transpose(2, 1, 0, 3)
            .reshape(NSLAB * P, AWS_)
        )
        # [el, t*2048 + ch*128 + p] = beta[t*128 + p, ch*126 + el]
        beta_t = np.ascontiguousarray(
            beta[sl].reshape(T, P, NCH, ECH).transpose(3, 0, 2, 1).reshape(ECH, -1)
        )
        maps.append(
            {
                "alpha_s": alpha_s,
                "beta_t": beta_t,
                "theta_t": np.ascontiguousarray(theta[sl].reshape(T, P).T),
                "mneg0": mneg0,
                "mneg1": mneg1,
                "mt_c": mt,
            }
        )
    return maps


def _run(alpha, beta, theta, **spmd_kwargs):
    from concourse.bass_utils import run_bass_kernel_spmd

    nc, _ = _get_compiled()
    res = run_bass_kernel_spmd(
        nc, _in_maps(alpha, beta, theta), core_ids=list(range(N_CORES)), **spmd_kwargs
    )
    total = np.float32(0.0)
    for r in res.results:
        total += np.float32(r["loss_part"][0, 0])
    return np.float32(total), res


def kernel(alpha, beta, theta):
    out, _ = _run(alpha, beta, theta)
    return out
